# revision 1
# baseline (speedup 1.0000x reference)
"""MBart expert-layer (MoE routing) kernel for 8 Trainium2 NeuronCores.

Strategy: data-parallel over batch. Each batch row routes to exactly one
expert (lang code), so the expert gather happens on host (langs are host
data).  Core b computes a dense SwiGLU MLP for row b:
    out = (gelu(x @ W1) * (x @ W3)) @ W2
All device work happens in transposed orientation (activations stored
[d_model, seq]) so both matmul stages use the natural [K, M] weight layouts
as the stationary operand and no on-device transposes are needed.
Matmul inputs are bf16 (fp32 accumulate in PSUM); gelu/mul in fp32.

v2 changes vs baseline (690.4us):
  - PE warm-up: a run of tiny matmuls on memset scratch issued before the
    real work so the HAM clock-gate reaches 8/8 (2.4GHz) while the first
    input DMAs are still in flight (baseline ran its first ~7us of real
    matmuls at 1.2GHz).
  - x is a single SBUF tile [P, S/MT, DT, MT] loaded by 4 chunk DMAs
    (1MB each) on the sync queue instead of 16 per-d-tile DMAs; chunk 0
    first so the first matmul chain unblocks earliest.  One tile tag
    instead of 8 also shortens the TileContext end-drain wait chain.
  - h is a single SBUF tile [P, FT, sg] (one tag instead of 32).
  - w3 tile DMA issued before w1 per f-tile (the b-path matmuls run
    first); weight pool deepened to 6 buffers.
  - w2 block loaded by one DMA per d-tile instead of 4 sub-DMAs.

The TPB ISA allows one sync wait per instruction and this walrus build
refuses multi-wait instructions, so the module is built as bacc.Bacc and
nc.compile() runs bacc's generate_event_semaphores pass, which splits
excess waits into event-semaphore chains.  build_nc() asserts the
resulting <=1-wait invariant.
"""

import numpy as np
import ml_dtypes
from contextlib import ExitStack

import concourse.bass as bass
import concourse.bacc as bacc
import concourse.mybir as mybir
from concourse.tile import TileContext
from concourse.bass_utils import run_bass_kernel_spmd

E, B, S, D, F = 8, 8, 2048, 1024, 4096
LANG_BASE = 4
P = 128
MT = 512          # matmul moving free dim (seq chunk)
NG = 2            # seq super-chunks; weights streamed NG times
NWARM = 76        # PE warm-up matmuls (N=128 each, ~110ns apiece): bridge
                  # PE busy-ness from ~7.1us (end of preamble) to ~14.5us
                  # (x chunk 0 arrival) so the HAM clock-gate stays
                  # released and the first real matmuls run at 2.4GHz
BF16 = mybir.dt.bfloat16
F32 = mybir.dt.float32
bf16 = ml_dtypes.bfloat16


def build_nc(S_=S, D_=D, F_=F, MT_=MT, NG_=NG, nwarm=NWARM,
             act=mybir.ActivationFunctionType.Gelu, check_waits=True):
    DT, FT = D_ // P, F_ // P
    sg = S_ // NG_
    nm = sg // MT_
    NC = S_ // MT_                    # total seq chunks
    nc = bacc.Bacc()
    xt = nc.declare_dram_parameter("xt", [P, NC, DT, MT_], BF16, isOutput=False)
    w1 = nc.declare_dram_parameter("w1", [FT, P, DT, P], BF16, isOutput=False)
    w3 = nc.declare_dram_parameter("w3", [FT, P, DT, P], BF16, isOutput=False)
    w2 = nc.declare_dram_parameter("w2", [DT, P, FT, P], BF16, isOutput=False)
    ot = nc.declare_dram_parameter("ot", [DT, P, S_], F32, isOutput=True)

    with TileContext(nc) as tc, ExitStack() as ctx:
        xpool = ctx.enter_context(tc.tile_pool(name="x", bufs=1))
        wpool = ctx.enter_context(tc.tile_pool(name="w", bufs=4))
        w2pool = ctx.enter_context(tc.tile_pool(name="w2", bufs=2))
        hpool = ctx.enter_context(tc.tile_pool(name="h", bufs=1))
        gpool = ctx.enter_context(tc.tile_pool(name="g", bufs=3))
        opool = ctx.enter_context(tc.tile_pool(name="o", bufs=3))
        ppool = ctx.enter_context(tc.tile_pool(name="ps", bufs=2, space="PSUM"))
        p2pool = ctx.enter_context(tc.tile_pool(name="ps2", bufs=2, space="PSUM"))
        wmpool = ctx.enter_context(tc.tile_pool(name="wm", bufs=1, space="PSUM"))

        # ---- PE warm-up: release the HAM clock throttle while input DMAs
        # are still streaming.  Scratch in, scratch out; nothing reads it.
        warm_sb = xpool.tile([P, P], BF16, name="warm_sb", tag="warm_sb")
        warm_ps = wmpool.tile([P, P], F32, name="warm_ps", tag="warm_ps")
        # memset on the (otherwise idle) vector engine: the gpsimd queue
        # must start issuing the critical first DMAs immediately.
        nc.vector.memset(warm_sb[:], 0.0)
        for _ in range(nwarm):
            nc.tensor.matmul(warm_ps[:], warm_sb[:], warm_sb[:],
                             start=True, stop=True)

        # ---- x: one tile, chunk-major, one DMA per chunk on the sync
        # hw-dge queue (per-queue transfers run serially in trigger order,
        # so chunk 0 — which gates the first matmul chain — transfers
        # first while the weight stream rides the gpsimd queue).
        x_sb = xpool.tile([P, NC, DT, MT_], BF16, name="x", tag="x")
        for c in range(NC):
            nc.sync.dma_start(out=x_sb[:, c], in_=xt[:, c])

        h_sb = hpool.tile([P, FT, sg], BF16, name="h", tag="h")

        def phase_a_fm(g, f_i, m, w3_t, w1_t):
            c = g * nm + m
            a_ps = ppool.tile([P, MT_], F32, name="a_ps", tag="a")
            b_ps = ppool.tile([P, MT_], F32, name="b_ps", tag="b")
            # b (w3 path) first so the gelu is the latest producer
            # feeding the h-mul: the wait legalizer can then anchor
            # the mul's PE wait on the gelu at zero cost.
            for d_i in range(DT):
                nc.tensor.matmul(
                    b_ps[:], w3_t[:, d_i, :], x_sb[:, c, d_i, :],
                    start=(d_i == 0), stop=(d_i == DT - 1))
            for d_i in range(DT):
                nc.tensor.matmul(
                    a_ps[:], w1_t[:, d_i, :], x_sb[:, c, d_i, :],
                    start=(d_i == 0), stop=(d_i == DT - 1))
            g_sb = gpool.tile([P, MT_], F32, name="g_sb", tag="g")
            nc.scalar.activation(g_sb[:], a_ps[:], act)
            nc.vector.tensor_mul(
                h_sb[:, f_i, m * MT_:(m + 1) * MT_], g_sb[:], b_ps[:])

        for g in range(NG_):
            # ---- phase A: hT[f, m] = gelu(W1.T x) * (W3.T x) ----
            for f_i in range(FT):
                w3_t = wpool.tile([P, DT, P], BF16, name="w3t", tag="w3t")
                w1_t = wpool.tile([P, DT, P], BF16, name="w1t", tag="w1t")
                nc.gpsimd.dma_start(out=w3_t[:], in_=w3[f_i])
                nc.gpsimd.dma_start(out=w1_t[:], in_=w1[f_i])
                for m in range(nm):
                    phase_a_fm(g, f_i, m, w3_t, w1_t)
            # ---- phase B: outT[d, m] = W2.T hT ----
            s0 = g * sg
            for d_i in range(DT):
                w2_t = w2pool.tile([P, FT, P], BF16, name="w2t", tag="w2t")
                nc.gpsimd.dma_start(out=w2_t[:], in_=w2[d_i])
                for m in range(nm):
                    o_ps = p2pool.tile([P, MT_], F32, name="o_ps", tag="o")
                    for f_i in range(FT):
                        nc.tensor.matmul(
                            o_ps[:], w2_t[:, f_i, :],
                            h_sb[:, f_i, m * MT_:(m + 1) * MT_],
                            start=(f_i == 0), stop=(f_i == FT - 1))
                    o_sb = opool.tile([P, MT_], F32, name="o_sb", tag="osb")
                    last = (g == NG_ - 1 and d_i == DT - 1 and m == nm - 1)
                    if last:
                        # Pipeline the final copy+store in halves so the
                        # end-of-kernel drain starts sooner.
                        hm = MT_ // 2
                        for k in range(2):
                            sl = slice(k * hm, (k + 1) * hm)
                            nc.vector.tensor_copy(o_sb[:, sl], o_ps[:, sl])
                            nc.sync.dma_start(
                                out=ot[d_i][:, s0 + m * MT_ + k * hm:
                                            s0 + m * MT_ + (k + 1) * hm],
                                in_=o_sb[:, sl])
                    else:
                        nc.vector.tensor_copy(o_sb[:], o_ps[:])
                        nc.sync.dma_start(
                            out=ot[d_i][:, s0 + m * MT_:s0 + (m + 1) * MT_],
                            in_=o_sb[:])

    nc.compile()
    if check_waits:
        skip = ("InstDrain", "InstEventSemaphore")
        bad = []
        for f in nc.m.functions:
            for bb in f.blocks:
                for inst in bb.instructions:
                    if type(inst).__name__ in skip or inst.sync_info is None:
                        continue
                    nw = len(inst.sync_info.on_wait or [])
                    if nw > 1:
                        bad.append((inst.name, type(inst).__name__, nw))
        if bad:
            raise RuntimeError(f"insts with >1 wait: {bad[:8]}")
    return nc


_NC_CACHE = {}


def _get_nc():
    if "nc" not in _NC_CACHE:
        _NC_CACHE["nc"] = build_nc()
    return _NC_CACHE["nc"]


def make_in_maps(hidden_states, w1, w2, w3, langs):
    hs = np.asarray(hidden_states, np.float32)
    w1 = np.asarray(w1, np.float32)
    w2 = np.asarray(w2, np.float32)
    w3 = np.asarray(w3, np.float32)
    langs = np.asarray(langs)
    DT, FT = D // P, F // P
    NC = S // MT
    in_maps = []
    for b in range(B):
        e = int(langs[b, 0] - LANG_BASE) % E
        # xt[p, c, d_i, j] = hs[c*MT+j, d_i*128+p]
        xtb = np.ascontiguousarray(
            hs[b].reshape(NC, MT, DT, P).transpose(3, 0, 2, 1).astype(bf16))
        w1b = np.ascontiguousarray(
            w1[e].reshape(DT, P, FT, P).transpose(2, 1, 0, 3).astype(bf16))
        w3b = np.ascontiguousarray(
            w3[e].reshape(DT, P, FT, P).transpose(2, 1, 0, 3).astype(bf16))
        w2b = np.ascontiguousarray(
            w2[e].reshape(FT, P, DT, P).transpose(2, 1, 0, 3).astype(bf16))
        in_maps.append({"xt": xtb, "w1": w1b, "w3": w3b, "w2": w2b})
    return in_maps


def assemble_output(results):
    out = np.empty((B, S, D), np.float32)
    for b in range(B):
        out[b] = results[b]["ot"].reshape(D, S).T
    return out


def kernel(hidden_states, w1, w2, w3, langs, **kw):
    nc = _get_nc()
    in_maps = make_in_maps(hidden_states, w1, w2, w3, langs)
    res = run_bass_kernel_spmd(nc, in_maps, list(range(8)))
    return assemble_output(res.results)


if __name__ == "__main__":
    rng = np.random.default_rng(0)
    hs = rng.standard_normal((B, S, D)).astype(np.float32)
    w1_ = (rng.standard_normal((E, D, F)) / np.sqrt(D)).astype(np.float32)
    w3_ = (rng.standard_normal((E, D, F)) / np.sqrt(D)).astype(np.float32)
    w2_ = (rng.standard_normal((E, F, D)) / np.sqrt(F)).astype(np.float32)
    langs = rng.integers(4, 12, (B, 1)).astype(np.int64)
    out = kernel(hs, w1_, w2_, w3_, langs)
    print(out.shape, out.dtype)



# revision 2
# speedup vs baseline: 1.1887x; 1.1887x over previous
"""MBart expert-layer (MoE routing) kernel for 8 Trainium2 NeuronCores.

Strategy: data-parallel over batch.  Each batch row routes to exactly one
expert (lang code), so the expert gather happens on host (langs are host
data).  Core b computes a dense SwiGLU MLP for row b:
    out = (gelu(x @ W1) * (x @ W3)) @ W2
All device work happens in transposed orientation (activations stored
[d_model, seq]) so both matmul stages use the natural [K, M] weight layouts
as the stationary operand and no on-device transposes are needed.
Matmul inputs are bf16 (fp32 accumulate in PSUM); gelu/mul in fp32.

The body runs at the PE cycle floor: 3072 matmuls x 512 moving rows,
back-to-back with zero structural stalls (verified by trace: median MM gap
= 512 PE cycles).  fp8 DoubleRow was measured at 2x/instruction but pure
e4m3 gives 7e-2 rel err vs the 2e-2 gate, and any hi/lo correction scheme
needs >=1.5 DR instructions per k-tile (slower than bf16), so bf16 is
optimal here.

v3 (this file) vs v2 (685-746us depending on the run's PE clock draw):
  - PE warm-up reads the (garbage) h tile instead of a memset scratch
    tile: no DVE dependency, so warm-up starts right after the PE queue
    preamble (~7.2us) and the HAM clock-gate reaches 8/8 by ~10.7us
    (was 12.7us).
  - Warm-up accumulates into the o-psum pool; a/b PSUM tiles share one
    6-buffer tag (deeper recycle distance); 8 PSUM banks total.
  - gelu and output-copy SBUF tiles share one 4-buffer pool; 8 tile tags
    instead of 12 shortens the end-of-kernel event-semaphore drain.

The TPB ISA allows one sync wait per instruction and this walrus build
refuses multi-wait instructions, so the module is built as bacc.Bacc and
nc.compile() runs bacc's generate_event_semaphores pass, which splits
excess waits into event-semaphore chains.  build_nc() asserts the
resulting <=1-wait invariant.
"""

import numpy as np
import ml_dtypes
from contextlib import ExitStack

import concourse.bass as bass
import concourse.bacc as bacc
import concourse.mybir as mybir
from concourse.tile import TileContext
from concourse.bass_utils import run_bass_kernel_spmd

E, B, S, D, F = 8, 8, 2048, 1024, 4096
LANG_BASE = 4
P = 128
MT = 512          # matmul moving free dim (seq chunk)
NG = 2            # seq super-chunks; weights streamed NG times
NWARM = 76        # PE warm-up matmuls (N=128 each, ~110ns apiece): bridge
                  # PE busy-ness from ~7.1us (end of preamble) to ~14.5us
                  # (x chunk 0 arrival) so the HAM clock-gate stays
                  # released and the first real matmuls run at 2.4GHz
BF16 = mybir.dt.bfloat16
F32 = mybir.dt.float32
bf16 = ml_dtypes.bfloat16


def build_nc(S_=S, D_=D, F_=F, MT_=MT, NG_=NG, nwarm=NWARM,
             act=mybir.ActivationFunctionType.Gelu, check_waits=True):
    DT, FT = D_ // P, F_ // P
    sg = S_ // NG_
    nm = sg // MT_
    NC = S_ // MT_                    # total seq chunks
    nc = bacc.Bacc()
    xt = nc.declare_dram_parameter("xt", [P, NC, DT, MT_], BF16, isOutput=False)
    w1 = nc.declare_dram_parameter("w1", [FT, P, DT, P], BF16, isOutput=False)
    w3 = nc.declare_dram_parameter("w3", [FT, P, DT, P], BF16, isOutput=False)
    w2 = nc.declare_dram_parameter("w2", [DT, P, FT, P], BF16, isOutput=False)
    ot = nc.declare_dram_parameter("ot", [DT, P, S_], F32, isOutput=True)

    with TileContext(nc) as tc, ExitStack() as ctx:
        xpool = ctx.enter_context(tc.tile_pool(name="x", bufs=1))
        wpool = ctx.enter_context(tc.tile_pool(name="w", bufs=4))
        w2pool = ctx.enter_context(tc.tile_pool(name="w2", bufs=2))
        hpool = ctx.enter_context(tc.tile_pool(name="h", bufs=1))
        vpool = ctx.enter_context(tc.tile_pool(name="v", bufs=4))
        ppool = ctx.enter_context(tc.tile_pool(name="ps", bufs=6, space="PSUM"))
        p2pool = ctx.enter_context(tc.tile_pool(name="ps2", bufs=2, space="PSUM"))

        h_sb = hpool.tile([P, FT, sg], BF16, name="h", tag="h")

        # ---- PE warm-up: release the HAM clock throttle while input DMAs
        # are still streaming.  Reads garbage h contents; result discarded.
        warm_ps = p2pool.tile([P, MT_], F32, name="warm_ps", tag="o")
        for _ in range(nwarm):
            nc.tensor.matmul(warm_ps[:, :P], h_sb[:, 0, :P], h_sb[:, 0, :P],
                             start=True, stop=True)

        # ---- x: one tile, chunk-major, one DMA per chunk on the sync
        # hw-dge queue (per-queue transfers run serially in trigger order,
        # so chunk 0 — which gates the first matmul chain — transfers
        # first while the weight stream rides the gpsimd queue).
        x_sb = xpool.tile([P, NC, DT, MT_], BF16, name="x", tag="x")
        for c in range(NC):
            nc.sync.dma_start(out=x_sb[:, c], in_=xt[:, c])

        def phase_a_fm(g, f_i, m, w3_t, w1_t):
            c = g * nm + m
            b_ps = ppool.tile([P, MT_], F32, name="b_ps", tag="ab")
            a_ps = ppool.tile([P, MT_], F32, name="a_ps", tag="ab")
            # b (w3 path) first so the gelu is the latest producer
            # feeding the h-mul: the wait legalizer can then anchor
            # the mul's PE wait on the gelu at zero cost.
            for d_i in range(DT):
                nc.tensor.matmul(
                    b_ps[:], w3_t[:, d_i, :], x_sb[:, c, d_i, :],
                    start=(d_i == 0), stop=(d_i == DT - 1))
            for d_i in range(DT):
                nc.tensor.matmul(
                    a_ps[:], w1_t[:, d_i, :], x_sb[:, c, d_i, :],
                    start=(d_i == 0), stop=(d_i == DT - 1))
            g_sb = vpool.tile([P, MT_], F32, name="g_sb", tag="v")
            nc.scalar.activation(g_sb[:], a_ps[:], act)
            nc.vector.tensor_mul(
                h_sb[:, f_i, m * MT_:(m + 1) * MT_], g_sb[:], b_ps[:])

        for g in range(NG_):
            # ---- phase A: hT[f, m] = gelu(W1.T x) * (W3.T x) ----
            for f_i in range(FT):
                w3_t = wpool.tile([P, DT, P], BF16, name="w3t", tag="w3t")
                w1_t = wpool.tile([P, DT, P], BF16, name="w1t", tag="w1t")
                nc.gpsimd.dma_start(out=w3_t[:], in_=w3[f_i])
                nc.gpsimd.dma_start(out=w1_t[:], in_=w1[f_i])
                for m in range(nm):
                    phase_a_fm(g, f_i, m, w3_t, w1_t)
            # ---- phase B: outT[d, m] = W2.T hT ----
            s0 = g * sg
            for d_i in range(DT):
                w2_t = w2pool.tile([P, FT, P], BF16, name="w2t", tag="w2t")
                nc.gpsimd.dma_start(out=w2_t[:], in_=w2[d_i])
                for m in range(nm):
                    o_ps = p2pool.tile([P, MT_], F32, name="o_ps", tag="o")
                    for f_i in range(FT):
                        nc.tensor.matmul(
                            o_ps[:], w2_t[:, f_i, :],
                            h_sb[:, f_i, m * MT_:(m + 1) * MT_],
                            start=(f_i == 0), stop=(f_i == FT - 1))
                    o_sb = vpool.tile([P, MT_], F32, name="o_sb", tag="v")
                    last = (g == NG_ - 1 and d_i == DT - 1 and m == nm - 1)
                    if last:
                        # Pipeline the final copy+store in halves so the
                        # end-of-kernel drain starts sooner.
                        hm = MT_ // 2
                        for k in range(2):
                            sl = slice(k * hm, (k + 1) * hm)
                            nc.vector.tensor_copy(o_sb[:, sl], o_ps[:, sl])
                            nc.sync.dma_start(
                                out=ot[d_i][:, s0 + m * MT_ + k * hm:
                                            s0 + m * MT_ + (k + 1) * hm],
                                in_=o_sb[:, sl])
                    else:
                        nc.vector.tensor_copy(o_sb[:], o_ps[:])
                        nc.sync.dma_start(
                            out=ot[d_i][:, s0 + m * MT_:s0 + (m + 1) * MT_],
                            in_=o_sb[:])

    nc.compile()
    if check_waits:
        skip = ("InstDrain", "InstEventSemaphore")
        bad = []
        for f in nc.m.functions:
            for bb in f.blocks:
                for inst in bb.instructions:
                    if type(inst).__name__ in skip or inst.sync_info is None:
                        continue
                    nw = len(inst.sync_info.on_wait or [])
                    if nw > 1:
                        bad.append((inst.name, type(inst).__name__, nw))
        if bad:
            raise RuntimeError(f"insts with >1 wait: {bad[:8]}")
    return nc


_NC_CACHE = {}


def _get_nc():
    if "nc" not in _NC_CACHE:
        _NC_CACHE["nc"] = build_nc()
    return _NC_CACHE["nc"]


def make_in_maps(hidden_states, w1, w2, w3, langs):
    hs = np.asarray(hidden_states, np.float32)
    w1 = np.asarray(w1, np.float32)
    w2 = np.asarray(w2, np.float32)
    w3 = np.asarray(w3, np.float32)
    langs = np.asarray(langs)
    DT, FT = D // P, F // P
    NC = S // MT
    in_maps = []
    for b in range(B):
        e = int(langs[b, 0] - LANG_BASE) % E
        # xt[p, c, d_i, j] = hs[c*MT+j, d_i*128+p]
        xtb = np.ascontiguousarray(
            hs[b].reshape(NC, MT, DT, P).transpose(3, 0, 2, 1).astype(bf16))
        w1b = np.ascontiguousarray(
            w1[e].reshape(DT, P, FT, P).transpose(2, 1, 0, 3).astype(bf16))
        w3b = np.ascontiguousarray(
            w3[e].reshape(DT, P, FT, P).transpose(2, 1, 0, 3).astype(bf16))
        w2b = np.ascontiguousarray(
            w2[e].reshape(FT, P, DT, P).transpose(2, 1, 0, 3).astype(bf16))
        in_maps.append({"xt": xtb, "w1": w1b, "w3": w3b, "w2": w2b})
    return in_maps


def assemble_output(results):
    out = np.empty((B, S, D), np.float32)
    for b in range(B):
        out[b] = results[b]["ot"].reshape(D, S).T
    return out


def kernel(hidden_states, w1, w2, w3, langs, **kw):
    nc = _get_nc()
    in_maps = make_in_maps(hidden_states, w1, w2, w3, langs)
    res = run_bass_kernel_spmd(nc, in_maps, list(range(8)))
    return assemble_output(res.results)


if __name__ == "__main__":
    rng = np.random.default_rng(0)
    hs = rng.standard_normal((B, S, D)).astype(np.float32)
    w1_ = (rng.standard_normal((E, D, F)) / np.sqrt(D)).astype(np.float32)
    w3_ = (rng.standard_normal((E, D, F)) / np.sqrt(D)).astype(np.float32)
    w2_ = (rng.standard_normal((E, F, D)) / np.sqrt(F)).astype(np.float32)
    langs = rng.integers(4, 12, (B, 1)).astype(np.int64)
    out = kernel(hs, w1_, w2_, w3_, langs)
    print(out.shape, out.dtype)

